# revision 18
# baseline (speedup 1.0000x reference)
"""LoRA wrapper layer (dense_mlp) on 8 Trainium2 NeuronCores.

y = x @ W^T + b + 2.0 * ((x @ lora_A^T) @ lora_B^T)

Strategy (final; 929us baseline -> ~772us, rel_err 1.78e-2 < 2e-2):
  * Host merges the rank-16 LoRA update into the weight
    (W_eff = W + 2*lora_B@lora_A); device does one GEMM + bias.
  * Token-parallel: core c owns tokens [c*2048,(c+1)*2048), computes
    all 4096 out-features (per-core DMA ~56 MiB, no HBM contention).
  * Mixed precision split-K: the first KF8=10 k-tiles (of 32) run as
    fp8-e4m3 DoubleRow matmuls (2 k-tiles per 512-col instruction =
    2x fp16 throughput); the remaining 22 in fp16.  All accumulate
    into one PSUM f32 group per output tile.
  * Global x64 weight scale (W fp8+fp16, bias; host divides the fp16
    output back): power-of-2 so fp16 error is unchanged, but the fp8
    quantization of W clears the e4m3 subnormal range (~12% less
    error), buying KF8=10 within the error budget.
  * x^T shard resident in SBUF (fp8 pair tiles + fp16 k-pair tiles);
    W^T streamed once per n-chunk as fine tiles (bufs=2 ring).
  * DMA issue order == consumption order (the Sync engine issues DMAs
    serially at ~0.9us each, so order and count matter); eviction DMAs
    go on the Scalar engine queue so they never queue behind inputs.
  * n-chunk 0 runs k-outer/mb-inner (PE streams behind the DMAs);
    n-chunks 1..7 run half-chunk phases: all 8 m-blocks' fp8 sweeps
    back-to-back (one fp16<->fp8 PE mode switch per half-chunk — the
    switch exposes the 256-col DR LDWEIGHTS), then per m-block fp16
    sweep + eviction, so evictions spread evenly and the PE never
    idles (zero mid-kernel gaps; no clock re-throttle).
  * Per-core: 3456 matmuls x 216.6ns (512-col stream + ~3.3ns NX
    dispatch) = 749us PE-busy, ~10us startup, ~11us fixed epilogue.
"""

import numpy as np

# ---- problem constants (hardcoded per harness contract) ----
B, S, D_IN, D_OUT = 4, 4096, 4096, 4096
M_TOT = B * S                   # 16384 tokens
N_CORES = 8
M_SHARD = M_TOT // N_CORES      # 2048 tokens per core
SCALING = 2.0
P = 128

KT = D_IN // P                  # 32 k-tiles total
KF8 = 10                        # k-tiles computed in fp8 (even; 10 -> 5 pairs)
KP8 = KF8 // 2                  # fp8 DoubleRow pairs
KP16 = (KT - KF8) // 2          # fp16 k-tile pairs (x tile granularity)
NCH = 8                         # n-chunks of 512 out-features
NW = 512                        # out-features per n-chunk (1 PSUM bank)
MB = M_SHARD // P               # 16 m-blocks per core

# Global weight scale: W (fp8 + fp16), bias are shipped *64 so the fp8
# e4m3 quantization of W stays clear of the subnormal range; the fp16
# output is y*64 and the host divides it back.  Powers of two are
# exact in fp16, so only the fp8 error improves (~12% lower).
WSCALE = 64.0

MM_DTYPE = "float16"
OUT_BUFS = 4

_cache = {}


def build_nc():
    from concourse import bacc, tile, mybir

    mm_dt = getattr(mybir.dt, MM_DTYPE)
    f8 = mybir.dt.float8e4
    f32 = mybir.dt.float32
    DR = mybir.MatmulPerfMode.DoubleRow

    nc = bacc.Bacc("TRN2", target_bir_lowering=False, debug=False)

    # fp8 x pairs: x8[kp*128+p, i, t] = x_c[t, (2kp+i)*128+p] fp8
    x8d = nc.dram_tensor("x8d", [KP8 * P, 2, M_SHARD], f8, kind="ExternalInput")
    # fp8 W pairs: w8d[kp*128+p, nch*1024 + i*512 + n] = W_eff[nch*512+n, (2kp+i)*128+p]
    w8d = nc.dram_tensor("w8d", [KP8 * P, NCH * 2 * NW], f8, kind="ExternalInput")
    # fp16 x k-pairs: xt[(kq*2+h)*128+p, i*1024+t] =
    #   x_c[h*1024+t, (KF8+2kq+i)*128+p]   (kq in [0,KP16), h half, i pair elt)
    xt = nc.dram_tensor("xt", [2 * KP16 * P, 2048], mm_dt, kind="ExternalInput")
    # fp16 W: wt[nch*128+p, ki*512+n] = W_eff[nch*512+n, (KF8+ki)*128+p]
    wt = nc.dram_tensor("wt", [NCH * P, (KT - KF8) * NW], mm_dt, kind="ExternalInput")
    bt = nc.dram_tensor("bt", [P, D_OUT], mm_dt, kind="ExternalInput")
    y = nc.dram_tensor("y", [M_SHARD, D_OUT], mm_dt, kind="ExternalOutput")

    with tile.TileContext(nc) as tc:
        with tc.tile_pool(name="const", bufs=1) as cpool, \
             tc.tile_pool(name="w", bufs=2) as wpool, \
             tc.tile_pool(name="out", bufs=OUT_BUFS) as opool, \
             tc.tile_pool(name="ps", bufs=8, space="PSUM") as pspool:

            bias = cpool.tile([P, D_OUT], mm_dt, name="bias")
            x8 = [cpool.tile([P, 2, M_SHARD], f8, name=f"x8_{kp}")
                  for kp in range(KP8)]
            xk = [cpool.tile([P, 2048], mm_dt, name=f"xk{t}")
                  for t in range(2 * KP16)]

            def dma_x8(kp, h):
                # half h covers tokens [h*1024,(h+1)*1024) = m-blocks 8h..8h+7
                nc.sync.dma_start(
                    out=x8[kp][:, :, h * 1024:(h + 1) * 1024],
                    in_=x8d[kp * P:(kp + 1) * P, :, h * 1024:(h + 1) * 1024])

            def dma_xk(kq, h):
                t = 2 * kq + h
                nc.sync.dma_start(out=xk[t][:],
                                  in_=xt[t * P:(t + 1) * P, :])

            def w8_tile(nch, kp):
                w = wpool.tile([P, 2, NW], f8, name=f"w8_{kp}")
                nc.sync.dma_start(
                    out=w[:],
                    in_=w8d[kp * P:(kp + 1) * P,
                            nch * 2 * NW:(nch + 1) * 2 * NW])
                return w

            def w16_tile(nch, ki):
                w = wpool.tile([P, NW], mm_dt, name=f"w{ki}")
                nc.sync.dma_start(
                    out=w[:],
                    in_=wt[nch * P:(nch + 1) * P, ki * NW:(ki + 1) * NW])
                return w

            # ---- startup DMAs, in consumption order ----
            w8c = []
            for kp in range(KP8):
                w8c.append(w8_tile(0, kp))
                dma_x8(kp, 0)                   # h0: m-blocks 0..7 (pass 1)
            w16c = []
            for ki in range(KT - KF8):
                w16c.append(w16_tile(0, ki))
                if ki % 2 == 0:
                    dma_xk(ki // 2, 0)          # h0 tile for this k-pair
                if ki == 10:
                    nc.sync.dma_start(out=bias[:], in_=bt[:, :])
            for kp in range(KP8):               # fp8 h1 (needed from ~55us)
                dma_x8(kp, 1)
            for kq in range(KP16):              # fp16 x h1
                dma_xk(kq, 1)
            w8n = [w8_tile(1, kp) for kp in range(KP8)]   # n-chunk 1 W
            w16n = [w16_tile(1, ki) for ki in range(KT - KF8)]

            def lhs16(ki, mb):
                # fp16 k-tile index ki in [0, KT-KF8)
                kq, i = divmod(ki, 2)
                h, m = divmod(mb, 8)
                col = i * 1024 + m * P
                return xk[2 * kq + h][:, col:col + P]

            def evict(ps, mb, nch):
                ot = opool.tile([P, NW], mm_dt, name="ot")
                nc.vector.tensor_add(ot[:], ps[:],
                                     bias[:, nch * NW:(nch + 1) * NW])
                nc.scalar.dma_start(
                    out=y[mb * P:(mb + 1) * P, nch * NW:(nch + 1) * NW],
                    in_=ot[:])

            def fp16_sweep(ps, mb, w16s):
                """fp16 remainder of the K accumulation (closes the group)."""
                for ki in range(KT - KF8):
                    nc.tensor.matmul(ps[:], lhsT=lhs16(ki, mb),
                                     rhs=w16s[ki][:],
                                     start=False, stop=(ki == KT - KF8 - 1))

            # ---- n-chunk 0: two k-outer passes (fast start) ----
            for hp in (0, 1):
                ps = [pspool.tile([P, NW], f32, name="ps") for _ in range(8)]
                for kp in range(KP8):
                    for m in range(8):
                        mb = hp * 8 + m
                        nc.tensor.matmul(ps[m][:],
                                         lhsT=x8[kp][:, :, mb * P:(mb + 1) * P],
                                         rhs=w8c[kp][:],
                                         start=(kp == 0), stop=False,
                                         perf_mode=DR)
                for ki in range(KT - KF8):
                    for m in range(8):
                        nc.tensor.matmul(ps[m][:], lhsT=lhs16(ki, hp * 8 + m),
                                         rhs=w16c[ki][:],
                                         start=False,
                                         stop=(ki == KT - KF8 - 1))
                for m in range(8):
                    evict(ps[m], hp * 8 + m, 0)

            # ---- n-chunks 1..7: half-chunk phases ----
            # All 8 m-blocks' fp8 DR sweeps run back-to-back (one
            # fp16<->fp8 PE mode switch per half-chunk instead of one
            # per tile; the switch exposes the 256-col DR LDWEIGHTS,
            # ~0.19us each), then each m-block's fp16 sweep + eviction
            # (evictions stay evenly spread).
            w8cur, w16cur = w8n, w16n
            for nch in range(1, NCH):
                w8nx, w16nx = [], []
                for half in (0, 1):
                    ps = [pspool.tile([P, NW], f32, name="ps")
                          for _ in range(8)]
                    for kp in range(KP8):
                        for m in range(8):
                            nc.tensor.matmul(
                                ps[m][:],
                                lhsT=x8[kp][:, :, (half * 8 + m) * P:
                                            (half * 8 + m + 1) * P],
                                rhs=w8cur[kp][:],
                                start=(kp == 0), stop=False,
                                perf_mode=DR)
                    for m in range(8):
                        mb = half * 8 + m
                        fp16_sweep(ps[m], mb, w16cur)
                        evict(ps[m], mb, nch)
                        # prefetch next chunk's W, 2 tiles per m-block
                        if nch < NCH - 1:
                            for j in range(2 * mb, min(2 * mb + 2,
                                                       KT - KF8 + KP8)):
                                if j < KP8:
                                    w8nx.append(w8_tile(nch + 1, j))
                                else:
                                    w16nx.append(w16_tile(nch + 1, j - KP8))
                w8cur, w16cur = w8nx, w16nx

    nc.compile()
    return nc


def prepare_in_maps(x, W, b, lora_A, lora_B):
    """Host-side prep: merge LoRA, pack/transpose/cast/quantize, shard."""
    import ml_dtypes
    mmdt = {"bfloat16": ml_dtypes.bfloat16,
            "float16": np.float16}[MM_DTYPE]
    e4 = ml_dtypes.float8_e4m3fn

    x2 = np.asarray(x, dtype=np.float32).reshape(M_TOT, D_IN)
    W_eff = np.asarray(W, dtype=np.float32) + SCALING * (
        np.asarray(lora_B, dtype=np.float32) @ np.asarray(lora_A, dtype=np.float32))
    W_eff = W_eff * WSCALE
    bf = np.asarray(b, dtype=np.float32) * WSCALE

    KF = KF8 * P                 # fp8 K columns
    # fp8 W pack: [kp, p, nch, i, n] <- W_eff[nch*512+n, (2kp+i)*128+p]
    w8 = W_eff[:, :KF].reshape(NCH, NW, KP8, 2, P).transpose(2, 4, 0, 3, 1)
    w8 = np.ascontiguousarray(w8.reshape(KP8 * P, NCH * 2 * NW)).astype(e4)
    # fp16 W pack: [nch, p, ki, n] <- W_eff[nch*512+n, (KF8+ki)*128+p]
    wp = W_eff[:, KF:].reshape(NCH, NW, KT - KF8, P).transpose(0, 3, 2, 1)
    wp = np.ascontiguousarray(
        wp.reshape(NCH * P, (KT - KF8) * NW)).astype(mmdt)

    bias = np.ascontiguousarray(np.broadcast_to(bf, (P, D_OUT))).astype(mmdt)

    in_maps = []
    for c in range(N_CORES):
        xc = x2[c * M_SHARD:(c + 1) * M_SHARD]          # [2048, 4096] f32
        # fp8 x pack: [kp, p, i, t] <- xc[t, (2kp+i)*128+p]
        x8 = xc[:, :KF].reshape(M_SHARD, KP8, 2, P).transpose(1, 3, 2, 0)
        x8 = np.ascontiguousarray(x8.reshape(KP8 * P, 2, M_SHARD)).astype(e4)
        # fp16 x pack: [kq, h, p, i, t] <- xc[h*1024+t, (KF8+2kq+i)*128+p]
        xf = xc[:, KF:].reshape(2, 1024, KP16, 2, P).transpose(2, 0, 4, 3, 1)
        xf = np.ascontiguousarray(
            xf.reshape(2 * KP16 * P, 2048)).astype(mmdt)
        in_maps.append({"x8d": x8, "w8d": w8, "xt": xf, "wt": wp, "bt": bias})
    return in_maps


def kernel(x, W, b, lora_A, lora_B):
    from concourse.bass_utils import run_bass_kernel_spmd

    key = ("nc", MM_DTYPE, KF8)
    if key not in _cache:
        _cache[key] = build_nc()
    nc = _cache[key]

    in_maps = prepare_in_maps(x, W, b, lora_A, lora_B)
    res = run_bass_kernel_spmd(nc, in_maps, list(range(N_CORES)))
    shards = [res.results[c]["y"] for c in range(N_CORES)]
    out = np.concatenate(shards, axis=0).astype(np.float32) * (1.0 / WSCALE)
    return np.ascontiguousarray(out.reshape(B, S, D_OUT))


# revision 22
# speedup vs baseline: 1.0672x; 1.0672x over previous
"""LoRA wrapper layer (dense_mlp) on 8 Trainium2 NeuronCores.

y = x @ W^T + b + 2.0 * ((x @ lora_A^T) @ lora_B^T)

Strategy (final; 929us baseline -> ~772us, rel_err 1.78e-2 < 2e-2):
  * Host merges the rank-16 LoRA update into the weight
    (W_eff = W + 2*lora_B@lora_A); device does one GEMM + bias.
  * Token-parallel: core c owns tokens [c*2048,(c+1)*2048), computes
    all 4096 out-features (per-core DMA ~56 MiB, no HBM contention).
  * Mixed precision split-K: the first KF8=10 k-tiles (of 32) run as
    fp8-e4m3 DoubleRow matmuls (2 k-tiles per 512-col instruction =
    2x fp16 throughput); the remaining 22 in fp16.  All accumulate
    into one PSUM f32 group per output tile.
  * Global x64 weight scale (W fp8+fp16, bias; host divides the fp16
    output back): power-of-2 so fp16 error is unchanged, but the fp8
    quantization of W clears the e4m3 subnormal range (~12% less
    error), buying KF8=10 within the error budget.
  * x^T shard resident in SBUF (fp8 pair tiles + fp16 k-pair tiles);
    W^T streamed once per n-chunk as fine tiles (bufs=2 ring).
  * DMA issue order == consumption order (the Sync engine issues DMAs
    serially at ~0.9us each, so order and count matter); eviction DMAs
    go on the Scalar engine queue so they never queue behind inputs.
  * n-chunk 0 runs k-outer/mb-inner (PE streams behind the DMAs);
    n-chunks 1..7 run mb-outer/k-inner so PSUM evictions spread evenly
    and the PE never idles (zero mid-kernel gaps; no re-throttle).
  * fp8 DR blocks stay short (<=1.7us) and interleaved with fp16 work:
    DR does 2x MACs/cycle, and long DR phases trip the PE activity/
    power throttle (observed +30..55us on throttled cores).
  * Per-core: 3456 matmuls x 216.6ns (512-col stream + ~3.3ns NX
    dispatch) = 749us PE-busy, ~10us startup, ~11us fixed epilogue.
"""

import numpy as np

# ---- problem constants (hardcoded per harness contract) ----
B, S, D_IN, D_OUT = 4, 4096, 4096, 4096
M_TOT = B * S                   # 16384 tokens
N_CORES = 8
M_SHARD = M_TOT // N_CORES      # 2048 tokens per core
SCALING = 2.0
P = 128

KT = D_IN // P                  # 32 k-tiles total
KF8 = 10                        # k-tiles computed in fp8 (even; 10 -> 5 pairs)
KP8 = KF8 // 2                  # fp8 DoubleRow pairs
KP16 = (KT - KF8) // 2          # fp16 k-tile pairs (x tile granularity)
NCH = 8                         # n-chunks of 512 out-features
NW = 512                        # out-features per n-chunk (1 PSUM bank)
MB = M_SHARD // P               # 16 m-blocks per core

# Global weight scale: W (fp8 + fp16), bias are shipped *64 so the fp8
# e4m3 quantization of W stays clear of the subnormal range; the fp16
# output is y*64 and the host divides it back.  Powers of two are
# exact in fp16, so only the fp8 error improves (~12% lower).
WSCALE = 64.0

MM_DTYPE = "float16"
OUT_BUFS = 4

_cache = {}


def build_nc():
    from concourse import bacc, tile, mybir

    mm_dt = getattr(mybir.dt, MM_DTYPE)
    f8 = mybir.dt.float8e4
    f32 = mybir.dt.float32
    DR = mybir.MatmulPerfMode.DoubleRow

    nc = bacc.Bacc("TRN2", target_bir_lowering=False, debug=False)

    # fp8 x pairs: x8[kp*128+p, i, t] = x_c[t, (2kp+i)*128+p] fp8
    x8d = nc.dram_tensor("x8d", [KP8 * P, 2, M_SHARD], f8, kind="ExternalInput")
    # fp8 W pairs: w8d[kp*128+p, nch*1024 + i*512 + n] = W_eff[nch*512+n, (2kp+i)*128+p]
    w8d = nc.dram_tensor("w8d", [KP8 * P, NCH * 2 * NW], f8, kind="ExternalInput")
    # fp16 x k-pairs: xt[(kq*2+h)*128+p, i*1024+t] =
    #   x_c[h*1024+t, (KF8+2kq+i)*128+p]   (kq in [0,KP16), h half, i pair elt)
    xt = nc.dram_tensor("xt", [2 * KP16 * P, 2048], mm_dt, kind="ExternalInput")
    # fp16 W: wt[nch*128+p, ki*512+n] = W_eff[nch*512+n, (KF8+ki)*128+p]
    wt = nc.dram_tensor("wt", [NCH * P, (KT - KF8) * NW], mm_dt, kind="ExternalInput")
    bt = nc.dram_tensor("bt", [P, D_OUT], mm_dt, kind="ExternalInput")
    y = nc.dram_tensor("y", [M_SHARD, D_OUT], mm_dt, kind="ExternalOutput")

    with tile.TileContext(nc) as tc:
        with tc.tile_pool(name="const", bufs=1) as cpool, \
             tc.tile_pool(name="w", bufs=2) as wpool, \
             tc.tile_pool(name="out", bufs=OUT_BUFS) as opool, \
             tc.tile_pool(name="ps", bufs=8, space="PSUM") as pspool:

            bias = cpool.tile([P, D_OUT], mm_dt, name="bias")
            x8 = [cpool.tile([P, 2, M_SHARD], f8, name=f"x8_{kp}")
                  for kp in range(KP8)]
            xk = [cpool.tile([P, 2048], mm_dt, name=f"xk{t}")
                  for t in range(2 * KP16)]

            def dma_x8(kp, h):
                # half h covers tokens [h*1024,(h+1)*1024) = m-blocks 8h..8h+7
                nc.sync.dma_start(
                    out=x8[kp][:, :, h * 1024:(h + 1) * 1024],
                    in_=x8d[kp * P:(kp + 1) * P, :, h * 1024:(h + 1) * 1024])

            def dma_xk(kq, h):
                t = 2 * kq + h
                nc.sync.dma_start(out=xk[t][:],
                                  in_=xt[t * P:(t + 1) * P, :])

            def w8_tile(nch, kp):
                w = wpool.tile([P, 2, NW], f8, name=f"w8_{kp}")
                nc.sync.dma_start(
                    out=w[:],
                    in_=w8d[kp * P:(kp + 1) * P,
                            nch * 2 * NW:(nch + 1) * 2 * NW])
                return w

            def w16_tile(nch, ki):
                w = wpool.tile([P, NW], mm_dt, name=f"w{ki}")
                nc.sync.dma_start(
                    out=w[:],
                    in_=wt[nch * P:(nch + 1) * P, ki * NW:(ki + 1) * NW])
                return w

            # ---- startup DMAs, in consumption order ----
            # chunk 0 pass 1 consumes interleaved: DR kp0, fp16 ki0-3,
            # DR kp1, fp16 ki4-7, ... so supply in the same order.
            w8c = [None] * KP8
            w16c = [None] * (KT - KF8)
            for b in range(KP8):
                w8c[b] = w8_tile(0, b)
                dma_x8(b, 0)                    # h0: m-blocks 0..7 (pass 1)
                for ki in range(4 * b, min(4 * b + 4, KT - KF8)):
                    w16c[ki] = w16_tile(0, ki)
                    if ki % 2 == 0:
                        dma_xk(ki // 2, 0)      # h0 tile for this k-pair
            for ki in range(4 * KP8, KT - KF8):
                w16c[ki] = w16_tile(0, ki)
                if ki % 2 == 0:
                    dma_xk(ki // 2, 0)
            nc.sync.dma_start(out=bias[:], in_=bt[:, :])
            for kp in range(KP8):               # fp8 h1 (needed from ~55us)
                dma_x8(kp, 1)
            for kq in range(KP16):              # fp16 x h1
                dma_xk(kq, 1)
            w8n = [w8_tile(1, kp) for kp in range(KP8)]   # n-chunk 1 W
            w16n = [w16_tile(1, ki) for ki in range(KT - KF8)]

            def lhs16(ki, mb):
                # fp16 k-tile index ki in [0, KT-KF8)
                kq, i = divmod(ki, 2)
                h, m = divmod(mb, 8)
                col = i * 1024 + m * P
                return xk[2 * kq + h][:, col:col + P]

            def evict(ps, mb, nch):
                ot = opool.tile([P, NW], mm_dt, name="ot")
                nc.vector.tensor_add(ot[:], ps[:],
                                     bias[:, nch * NW:(nch + 1) * NW])
                nc.scalar.dma_start(
                    out=y[mb * P:(mb + 1) * P, nch * NW:(nch + 1) * NW],
                    in_=ot[:])

            def fp16_sweep(ps, mb, w16s):
                """fp16 remainder of the K accumulation (closes the group)."""
                for ki in range(KT - KF8):
                    nc.tensor.matmul(ps[:], lhsT=lhs16(ki, mb),
                                     rhs=w16s[ki][:],
                                     start=False, stop=(ki == KT - KF8 - 1))

            # ---- n-chunk 0: two k-outer passes (fast start) ----
            # DR kp-blocks interleaved among fp16 ki-blocks (short 2x-MAC
            # bursts, see the power note below) in DMA-arrival order.
            def c0_dr_block(ps, hp, kp):
                for m in range(8):
                    mb = hp * 8 + m
                    nc.tensor.matmul(ps[m][:],
                                     lhsT=x8[kp][:, :, mb * P:(mb + 1) * P],
                                     rhs=w8c[kp][:],
                                     start=(kp == 0), stop=False,
                                     perf_mode=DR)

            def c0_fp16_block(ps, hp, ki):
                for m in range(8):
                    nc.tensor.matmul(ps[m][:], lhsT=lhs16(ki, hp * 8 + m),
                                     rhs=w16c[ki][:],
                                     start=False,
                                     stop=(ki == KT - KF8 - 1))

            for hp in (0, 1):
                ps = [pspool.tile([P, NW], f32, name="ps") for _ in range(8)]
                for b in range(KP8):
                    c0_dr_block(ps, hp, b)
                    for ki in range(4 * b, min(4 * b + 4, KT - KF8)):
                        c0_fp16_block(ps, hp, ki)
                for ki in range(4 * KP8, KT - KF8):
                    c0_fp16_block(ps, hp, ki)
                for m in range(8):
                    evict(ps[m], hp * 8 + m, 0)

            # ---- n-chunks 1..7: mb-outer / k-inner ----
            # Per-tile fp8 block (5 DR matmuls, ~1.1us) interleaved with
            # the fp16 sweep: short 2x-MAC power bursts stay under the
            # hardware activity-throttle window (grouping them into
            # ~9us DR phases trips the PE util limiter under marginal
            # power conditions: observed +30..55us on throttled cores,
            # while buying <1us of busy time).  Evictions spread evenly.
            w8cur, w16cur = w8n, w16n
            for nch in range(1, NCH):
                w8nx, w16nx = [], []
                for mb in range(MB):
                    ps = pspool.tile([P, NW], f32, name="ps")
                    for kp in range(KP8):
                        nc.tensor.matmul(
                            ps[:],
                            lhsT=x8[kp][:, :, mb * P:(mb + 1) * P],
                            rhs=w8cur[kp][:],
                            start=(kp == 0), stop=False,
                            perf_mode=DR)
                    fp16_sweep(ps, mb, w16cur)
                    evict(ps, mb, nch)
                    # prefetch next chunk's W, 2 tiles per m-block
                    if nch < NCH - 1:
                        for j in range(2 * mb, min(2 * mb + 2,
                                                   KT - KF8 + KP8)):
                            if j < KP8:
                                w8nx.append(w8_tile(nch + 1, j))
                            else:
                                w16nx.append(w16_tile(nch + 1, j - KP8))
                w8cur, w16cur = w8nx, w16nx

    nc.compile()
    return nc


def prepare_in_maps(x, W, b, lora_A, lora_B):
    """Host-side prep: merge LoRA, pack/transpose/cast/quantize, shard."""
    import ml_dtypes
    mmdt = {"bfloat16": ml_dtypes.bfloat16,
            "float16": np.float16}[MM_DTYPE]
    e4 = ml_dtypes.float8_e4m3fn

    x2 = np.asarray(x, dtype=np.float32).reshape(M_TOT, D_IN)
    W_eff = np.asarray(W, dtype=np.float32) + SCALING * (
        np.asarray(lora_B, dtype=np.float32) @ np.asarray(lora_A, dtype=np.float32))
    W_eff = W_eff * WSCALE
    bf = np.asarray(b, dtype=np.float32) * WSCALE

    KF = KF8 * P                 # fp8 K columns
    # fp8 W pack: [kp, p, nch, i, n] <- W_eff[nch*512+n, (2kp+i)*128+p]
    w8 = W_eff[:, :KF].reshape(NCH, NW, KP8, 2, P).transpose(2, 4, 0, 3, 1)
    w8 = np.ascontiguousarray(w8.reshape(KP8 * P, NCH * 2 * NW)).astype(e4)
    # fp16 W pack: [nch, p, ki, n] <- W_eff[nch*512+n, (KF8+ki)*128+p]
    wp = W_eff[:, KF:].reshape(NCH, NW, KT - KF8, P).transpose(0, 3, 2, 1)
    wp = np.ascontiguousarray(
        wp.reshape(NCH * P, (KT - KF8) * NW)).astype(mmdt)

    bias = np.ascontiguousarray(np.broadcast_to(bf, (P, D_OUT))).astype(mmdt)

    in_maps = []
    for c in range(N_CORES):
        xc = x2[c * M_SHARD:(c + 1) * M_SHARD]          # [2048, 4096] f32
        # fp8 x pack: [kp, p, i, t] <- xc[t, (2kp+i)*128+p]
        x8 = xc[:, :KF].reshape(M_SHARD, KP8, 2, P).transpose(1, 3, 2, 0)
        x8 = np.ascontiguousarray(x8.reshape(KP8 * P, 2, M_SHARD)).astype(e4)
        # fp16 x pack: [kq, h, p, i, t] <- xc[h*1024+t, (KF8+2kq+i)*128+p]
        xf = xc[:, KF:].reshape(2, 1024, KP16, 2, P).transpose(2, 0, 4, 3, 1)
        xf = np.ascontiguousarray(
            xf.reshape(2 * KP16 * P, 2048)).astype(mmdt)
        in_maps.append({"x8d": x8, "w8d": w8, "xt": xf, "wt": wp, "bt": bias})
    return in_maps


def kernel(x, W, b, lora_A, lora_B):
    from concourse.bass_utils import run_bass_kernel_spmd

    key = ("nc", MM_DTYPE, KF8)
    if key not in _cache:
        _cache[key] = build_nc()
    nc = _cache[key]

    in_maps = prepare_in_maps(x, W, b, lora_A, lora_B)
    res = run_bass_kernel_spmd(nc, in_maps, list(range(N_CORES)))
    shards = [res.results[c]["y"] for c in range(N_CORES)]
    out = np.concatenate(shards, axis=0).astype(np.float32) * (1.0 / WSCALE)
    return np.ascontiguousarray(out.reshape(B, S, D_OUT))
